# revision 58
# baseline (speedup 1.0000x reference)
"""Tree-GRU (arity-8, depth-5) over embedded leaves on 8 TRN2 NeuronCores.

Sharding: data-parallel over subtrees. Each core takes 4096 contiguous leaves
and runs levels 5 and 4 of the tree locally (512 -> 64 parents). The last two
per-core levels (64 -> 8 -> 1) and the root are small latency-bound GRU
cascades (free dim <= 8) done on host in fp64 after gathering the per-core
level-4 outputs, extending the baseline's host-side root reduction.

Device layout is feature-transposed: tensors live as [128 part, 3 ktile, ...]
with feature f = 128*k + p, so the GRU matmuls contract the partition dim.

Embeddings arrive per GRU step: tokens are host-permuted child-major, each
child's 512 rows fetched by 4 indirect DMAs (leaf-major) and flipped
feature-major by 4 xbar transpose-DMAs on the HWDGE rings — no tensor-engine
transposes, no PSUM, and the first GRU matmul can start after ~2 gathers.

Level 512 keeps one PSUM bank per (role, jo) output tile at N=512; each step
emits gi matmuls of units j0/j1 ahead of any hh matmul so the tensor engine
holds ~3.8us of h-independent work to hide the previous step's gate chain.
Unit j1 owns 4 banks (double-buffered step to step); j0 and j2 share the
other 4, with j2's allocation waiting on j0's progressively-freed banks
behind hh j1. Biases ride the scalar-activation bias port. The per-step
output accumulator is kept child-major (vector engine) so the final step
writes level 4's input directly as a fused raw-sum add; the 1/8 output-mean
scale is folded into a pre-scaled copy of W_ih used by level 4. Level 64
injects biases into PSUM via a K=3 one-hot matmul (the only start=True
write), collapsing the gate chain to jo-spanning instructions.
"""

import numpy as np
import ml_dtypes

ARITY = 8
DIM = 384
VOCAB = 32000
NCORES = 8
P = 128
J = 3  # DIM // 128 feature tiles
N_LEAVES = 32768
LEAVES_CORE = N_LEAVES // NCORES  # 4096
P5 = LEAVES_CORE // ARITY  # 512 level-5 parents per core
P4 = P5 // ARITY  # 64 level-4 parents per core
GT = P5 // P  # 4 gather tiles per child

BF16 = ml_dtypes.bfloat16

_PROG_CACHE = {}


def _emit(tc, nc, aps):
    import concourse.mybir as mybir
    import concourse.bass as bass
    from concourse.masks import make_identity

    f32 = mybir.dt.float32
    bf16 = mybir.dt.bfloat16
    Sig = mybir.ActivationFunctionType.Sigmoid
    Tanh = mybir.ActivationFunctionType.Tanh
    Add = mybir.AluOpType.add
    Sub = mybir.AluOpType.subtract
    Mult = mybir.AluOpType.mult

    tokens32, embed, wih_t, biases, biases_mm, bpack6, out_hacc, out_hf = aps

    from contextlib import ExitStack

    with ExitStack() as ctx:
        const = ctx.enter_context(tc.tile_pool(name="const", bufs=1))
        xpool = ctx.enter_context(tc.tile_pool(name="xpool", bufs=1))
        gpool = ctx.enter_context(tc.tile_pool(name="gpool", bufs=3))
        state = ctx.enter_context(tc.tile_pool(name="state", bufs=1))
        gates = ctx.enter_context(tc.tile_pool(name="gates", bufs=4))
        pspool = ctx.enter_context(tc.tile_pool(name="pspool", bufs=4, space="PSUM"))
        pspool2 = ctx.enter_context(tc.tile_pool(name="pspool2", bufs=4, space="PSUM"))

        # ---- token tile first, then per-child embedding gathers ----
        # all children are fetched leaf-major by native indirect DMAs (no
        # GpSimd library, starts right after the token DMA) and flipped
        # feature-major by tensor-engine transposes two steps ahead of use.
        tok32_sb = const.tile([P, ARITY * GT], mybir.dt.int32)
        nc.sync.dma_start(tok32_sb[:], tokens32[:])

        wpack_sb = const.tile([P, 3, J, 9, P], bf16)
        wih_sb = wpack_sb[:, 0]
        whh_sb = wpack_sb[:, 1]
        wih_s_sb = wpack_sb[:, 2]
        bias_sb = const.tile([P, 12], f32)
        bpack_sb = const.tile([3, 4 * P + 3 * 512], bf16)
        bias3_sb = bpack_sb[:, : 4 * P].rearrange("k (r p) -> k r p", r=4)
        onehot3_sb = bpack_sb[:, 4 * P :].rearrange("k (j n) -> k j n", j=3)
        bpack6_sb = const.tile([6, P + 6 * P4], bf16)
        bias6_sb = bpack6_sb[:, :P]
        onehot6_sb = bpack6_sb[:, P:].rearrange("k (g n) -> k g n", g=6)
        nc.sync.dma_start(wpack_sb[:], wih_t[:])
        nc.sync.dma_start(bias_sb[:], biases[:])
        nc.sync.dma_start(bpack_sb[:], biases_mm[:])
        nc.sync.dma_start(bpack6_sb[:], bpack6[:])

        # x5[p, child, j, q]
        x5 = xpool.tile([P, ARITY, J, P5], bf16, name="x5", tag="x5")
        ident = const.tile([P, P], bf16)
        make_identity(nc, ident[:])

        xgs = {}
        for t in range(ARITY):
            c = ARITY - 1 - t  # children consumed in reverse: child 7 first
            xg = gpool.tile([P, GT, DIM], bf16, name="xg", tag="xg")
            xgs[c] = xg
            for g in range(GT):
                gi_inst = nc.gpsimd.indirect_dma_start(
                    out=xg[:, g, :],
                    out_offset=None,
                    in_=embed[:],
                    in_offset=bass.IndirectOffsetOnAxis(
                        ap=tok32_sb[:, c * GT + g : c * GT + g + 1], axis=0
                    ),
                )
                if (t * GT + g) % 2 == 1:
                    gi_inst.ins.queue = "qPoolDynamic1"

        def emit_transposes(c):
            # 4 gather tiles of one feature third -> one PSUM bank, one copy
            xg = xgs[c]
            for j in range(J):
                tp = pspool2.tile([P, 512], bf16, name="tp", tag="ps2")
                for g in range(GT):
                    nc.tensor.transpose(
                        tp[:, g * P : (g + 1) * P],
                        xg[:, g, j * P : (j + 1) * P],
                        ident[:],
                    )
                nc.vector.tensor_copy(out=x5[:, c, j, :], in_=tp[:])

        emit_transposes(7)
        emit_transposes(6)

        x4 = xpool.tile([P, ARITY, J, P4], bf16, name="x4", tag="x4")

        def psum_tile(jo):
            # 8 banks for 12 role-tiles per step: unit j1 owns pspool (reuse
            # waits on the previous step's j1 gates); j0/j2 share pspool2 —
            # j2 waits on same-step j0 gates (freed progressively under hh
            # j1), j0 on the previous step's j2 gates. All waits point at
            # strictly earlier FIFO positions: no deadlock.
            if jo == 1:
                return pspool.tile([P, 512], f32, name="ps", tag="ps")
            return pspool2.tile([P, 512], f32, name="ps2", tag="ps2")

        # =================== level 5: 512 parents, leaf children ===================
        h5 = state.tile([P, J, P5], bf16, name="h5", tag="h5")
        hacc5 = state.tile([P, J, ARITY, P4], f32, name="hacc5", tag="hacc5")
        nc.gpsimd.memset(hacc5[:], 0.0)
        csum5 = state.tile([P, J, P4], f32, name="csum5", tag="csum5")

        with nc.named_scope("level_512"):
            for t in range(ARITY):
                c = ARITY - 1 - t
                leaf0 = t == 0

                ps_r = [None] * J
                ps_z = [None] * J
                ps_in = [None] * J
                ps_hn = [None] * J

                def emit_gi(jo):
                    ps_r[jo] = psum_tile(jo)
                    ps_z[jo] = psum_tile(jo)
                    ps_in[jo] = psum_tile(jo)
                    if not leaf0:
                        ps_hn[jo] = psum_tile(jo)
                    for ps, moff in ((ps_r[jo], 0), (ps_z[jo], 3), (ps_in[jo], 6)):
                        for ji in range(J):
                            nc.tensor.matmul(
                                ps[:, :P5],
                                wih_sb[:, ji, moff + jo, :],
                                x5[:, c, ji, :],
                                start=(ji == 0),
                                stop=(ji == 2 and (moff == 6 or leaf0)),
                            )

                def emit_hh(jo):
                    if leaf0:
                        return
                    for ps, moff in ((ps_r[jo], 0), (ps_z[jo], 3), (ps_hn[jo], 6)):
                        for ji in range(J):
                            nc.tensor.matmul(
                                ps[:, :P5],
                                whh_sb[:, ji, moff + jo, :],
                                h5[:, ji, :],
                                start=(ji == 0 and moff == 6),
                                stop=(ji == 2),
                            )

                emit_gi(0)
                emit_gi(1)
                if t <= 5:
                    # next-next child's feature flip rides behind ~3.8us of
                    # gi matmuls so its PSUM-bank wait never stalls the PE
                    emit_transposes(5 - t)
                emit_hh(0)
                emit_hh(1)
                emit_gi(2)
                emit_hh(2)

                for jo in range(J):
                    r_sb = gates.tile([P, P5], bf16, name="r_sb", tag="r_sb")
                    z_sb = gates.tile([P, P5], bf16, name="z_sb", tag="z_sb")
                    n_sb = gates.tile([P, P5], bf16, name="n_sb", tag="n_sb")
                    rhn = gates.tile([P, P5], f32, name="rhn", tag="rhn")
                    t1 = gates.tile([P, P5], bf16, name="t1", tag="t1")

                    nc.scalar.activation(
                        r_sb[:], ps_r[jo][:, :P5], Sig, bias=bias_sb[:, jo : jo + 1]
                    )
                    nc.scalar.activation(
                        z_sb[:], ps_z[jo][:, :P5], Sig, bias=bias_sb[:, 3 + jo : 4 + jo]
                    )
                    if leaf0:
                        nc.vector.tensor_scalar_mul(
                            rhn[:], r_sb[:], bias_sb[:, 6 + jo : 7 + jo]
                        )
                    else:
                        nc.vector.scalar_tensor_tensor(
                            out=rhn[:],
                            in0=ps_hn[jo][:, :P5],
                            scalar=bias_sb[:, 6 + jo : 7 + jo],
                            in1=r_sb[:],
                            op0=Add,
                            op1=Mult,
                        )
                    nc.vector.tensor_tensor(
                        out=rhn[:], in0=rhn[:], in1=ps_in[jo][:, :P5], op=Add
                    )
                    nc.scalar.activation(
                        n_sb[:], rhn[:], Tanh, bias=bias_sb[:, 9 + jo : 10 + jo]
                    )
                    hsl = h5[:, jo, :]
                    if leaf0:
                        nc.vector.tensor_tensor(out=t1[:], in0=z_sb[:], in1=n_sb[:], op=Mult)
                        nc.vector.tensor_tensor(out=hsl, in0=n_sb[:], in1=t1[:], op=Sub)
                    else:
                        nc.vector.tensor_tensor(out=t1[:], in0=hsl, in1=n_sb[:], op=Sub)
                        nc.vector.tensor_tensor(out=t1[:], in0=z_sb[:], in1=t1[:], op=Mult)
                        nc.vector.tensor_tensor(out=hsl, in0=n_sb[:], in1=t1[:], op=Add)

                # output accumulation after the chain ops so it never delays
                # the next step's recurrent matmuls
                for jo in range(J):
                    hsl = h5[:, jo, :]
                    hperm = hsl.rearrange("p (q c) -> p c q", c=ARITY)
                    if t == ARITY - 1:
                        nc.vector.tensor_reduce(
                            out=csum5[:, jo, :],
                            in_=hsl.rearrange("p (q c) -> p q c", c=ARITY),
                            axis=mybir.AxisListType.X,
                            op=Add,
                        )
                        # per-child writes, child 7 first, so level 64's
                        # first gi matmuls unblock before the whole add
                        for cc in range(ARITY - 1, -1, -1):
                            nc.vector.tensor_tensor(
                                out=x4[:, cc, jo, :],
                                in0=hacc5[:, jo, cc],
                                in1=hperm[:, cc],
                                op=Add,
                            )
                    else:
                        nc.vector.tensor_tensor(
                            out=hacc5[:, jo], in0=hacc5[:, jo], in1=hperm, op=Add
                        )

        # =================== level 4: 64 parents ===================
        h4 = state.tile([P, J, P4], bf16, name="h4", tag="h4")
        nc.scalar.mul(h4[:], csum5[:], 1.0 / ARITY)
        hacc4 = state.tile([P, J, P4], f32, name="hacc4", tag="hacc4")
        nc.gpsimd.memset(hacc4[:], 0.0)
        N3 = J * P4  # 192

        with nc.named_scope("level_64"):
            for t in range(ARITY):
                c = ARITY - 1 - t
                pool = pspool if t % 2 == 0 else pspool2
                tag = "ps" if t % 2 == 0 else "ps2"
                ps_rz, ps_hn, ps_in = (
                    pool.tile([P, 512], f32, name=tag, tag=tag) for _ in range(3)
                )

                def view3(pst):
                    return pst[:, :N3].rearrange("p (j n) -> p j n", j=3)

                # bias matmuls: the only start=True writes. r and z share one
                # tile via a K=6 one-hot so a single sigmoid covers both.
                nc.tensor.matmul(
                    ps_rz[:, : 2 * N3],
                    bias6_sb[:],
                    onehot6_sb[:],
                    start=True,
                    stop=False,
                )
                for pst, ro in ((ps_hn, 2), (ps_in, 3)):
                    nc.tensor.matmul(
                        pst[:, :N3],
                        bias3_sb[:, ro, :],
                        onehot3_sb[:, :, :P4],
                        start=True,
                        stop=False,
                    )
                for off, moff in ((0, 0), (N3, 3), (None, 6)):
                    pst, base = (ps_in, 0) if off is None else (ps_rz, off)
                    for jo in range(J):
                        for ji in range(J):
                            nc.tensor.matmul(
                                pst[:, base + jo * P4 : base + (jo + 1) * P4],
                                wih_s_sb[:, ji, moff + jo, :],
                                x4[:, c, ji, :],
                                start=False,
                                stop=(moff == 6 and jo == 2 and ji == 2),
                            )
                for off, moff in ((0, 0), (N3, 3), (None, 6)):
                    pst, base = (ps_hn, 0) if off is None else (ps_rz, off)
                    for jo in range(J):
                        for ji in range(J):
                            nc.tensor.matmul(
                                pst[:, base + jo * P4 : base + (jo + 1) * P4],
                                whh_sb[:, ji, moff + jo, :],
                                h4[:, ji, :],
                                start=False,
                                stop=(jo == 2 and ji == 2 and moff != 0),
                            )

                rz_sb = gates.tile([P, 2, J, P4], bf16, name="rz4", tag="rz4")
                n_sb = gates.tile([P, J, P4], bf16, name="n4", tag="n4")
                rhn = gates.tile([P, J, P4], f32, name="rhn4", tag="rhn4")
                t1 = gates.tile([P, J, P4], bf16, name="t14", tag="t14")
                r_sb = rz_sb[:, 0]
                z_sb = rz_sb[:, 1]

                nc.scalar.activation(
                    rz_sb[:],
                    ps_rz[:, : 2 * N3].rearrange("p (r j n) -> p r j n", r=2, j=3),
                    Sig,
                )
                nc.vector.tensor_tensor(
                    out=rhn[:], in0=view3(ps_hn), in1=r_sb, op=Mult
                )
                nc.vector.tensor_tensor(
                    out=rhn[:], in0=rhn[:], in1=view3(ps_in), op=Add
                )
                nc.scalar.activation(n_sb[:], rhn[:], Tanh)
                nc.vector.tensor_tensor(out=t1[:], in0=h4[:], in1=n_sb[:], op=Sub)
                nc.vector.tensor_tensor(out=t1[:], in0=z_sb, in1=t1[:], op=Mult)
                nc.vector.tensor_tensor(out=h4[:], in0=n_sb[:], in1=t1[:], op=Add)
                # off the critical chain: gpsimd is idle during level 64
                nc.gpsimd.tensor_tensor(
                    out=hacc4[:], in0=hacc4[:], in1=h4[:], op=Add
                )

        # ---- outputs: raw h-sum (x3*8) and final hiddens of the 64 nodes ----
        nc.sync.dma_start(out_hacc[:], hacc4[:])
        nc.sync.dma_start(out_hf[:], h4[:])


def _build_program():
    if "prog" in _PROG_CACHE:
        return _PROG_CACHE["prog"]
    import concourse.bacc as bacc
    import concourse.mybir as mybir
    import concourse.tile as tile

    f32 = mybir.dt.float32
    bf16 = mybir.dt.bfloat16

    nc = bacc.Bacc(
        "TRN2",
        target_bir_lowering=False,
        debug=False,
        enable_asserts=False,
        num_devices=NCORES,
        num_swdge_queues=2,
    )
    tokens32 = nc.dram_tensor(
        "tokens32", [P, ARITY * GT], mybir.dt.int32, kind="ExternalInput"
    ).ap()
    embed = nc.dram_tensor("embed", [VOCAB, DIM], bf16, kind="ExternalInput").ap()
    wpack = nc.dram_tensor("wpack", [P, 3, J, 9, P], bf16, kind="ExternalInput").ap()
    biases = nc.dram_tensor("biases", [P, 12], f32, kind="ExternalInput").ap()
    bpack = nc.dram_tensor(
        "bpack", [3, 4 * P + 3 * 512], bf16, kind="ExternalInput"
    ).ap()
    bpack6 = nc.dram_tensor(
        "bpack6", [6, P + 6 * P4], bf16, kind="ExternalInput"
    ).ap()
    out_hacc = nc.dram_tensor("out_hacc", [P, J, P4], f32, kind="ExternalOutput").ap()
    out_hf = nc.dram_tensor("out_hf", [P, J, P4], bf16, kind="ExternalOutput").ap()

    with tile.TileContext(nc) as tc:
        _emit(tc, nc, (tokens32, embed, wpack, biases, bpack, bpack6, out_hacc, out_hf))
    nc.compile()
    _PROG_CACHE["prog"] = nc
    return nc


def _retile_weights(w):
    # w: [1152, 384] -> lhsT tiles [128(k_part), 3(k), 9(m), 128(m_col)] bf16
    wt = np.ascontiguousarray(w.T)  # [384, 1152]
    wt = wt.reshape(J, P, 9, P).transpose(1, 0, 2, 3)
    return np.ascontiguousarray(wt).astype(BF16)


def _prep_bias(b_ih, b_hh):
    biases = np.zeros((P, 12), np.float32)
    comb = (b_ih + b_hh).reshape(9, P)
    biases[:, 0:6] = comb[0:6].T
    biases[:, 6:9] = b_hh.reshape(9, P)[6:9].T
    biases[:, 9:12] = b_ih.reshape(9, P)[6:9].T
    return biases


def _prep_bias_mm(b_ih, b_hh):
    # lhsT[k, ro, q] = bias[q, 3*ro + k]: the K=3 bias matmul against the
    # one-hot rhs yields out[q, (j, n)] = bias[q, 3*ro + j].
    b = _prep_bias(b_ih, b_hh)  # [128, 12] cols: r0..2 z0..2 hn0..2 in0..2
    out = b.T.reshape(4, 3, P).transpose(1, 0, 2)
    return np.ascontiguousarray(out).astype(BF16)


def _prep_onehot3():
    out = np.zeros((3, 3, 512), np.float32)
    for k in range(3):
        out[k, k, :] = 1.0
    return out.astype(BF16)


def _prep_bpack6(b_ih, b_hh):
    # K=6 bias matmul for the merged r+z PSUM tile: lhsT rows are the six
    # combined bias vectors (r jo0..2, z jo0..2), rhs is a [6, 6, 64] one-hot.
    b = _prep_bias(b_ih, b_hh)  # [128, 12]
    lhs = b[:, 0:6].T.astype(np.float32)  # [6, 128]
    oh = np.zeros((6, 6, P4), np.float32)
    for k in range(6):
        oh[k, k, :] = 1.0
    out = np.concatenate([lhs, oh.reshape(6, 6 * P4)], axis=1)
    return np.ascontiguousarray(out).astype(BF16)


def _prep_tokens32(tokens_core):
    # int32 indirect-DMA tokens: col c*4+g, row p holds tokens[(g*128+p)*8+c]
    tok = tokens_core.reshape(P5, ARITY).T  # [8 child, 512 parent]
    sel = tok.reshape(ARITY, GT, P).transpose(2, 0, 1).reshape(P, ARITY * GT)
    return np.ascontiguousarray(sel)


def _gru_level(x_children, h0, w_ih, w_hh, b_ih, b_hh):
    # x_children: [A, N, D] in original child order; consumed reversed.
    h = h0
    acc = np.zeros_like(h)
    for t in range(ARITY):
        x_t = x_children[ARITY - 1 - t]
        gi = x_t @ w_ih.T + b_ih
        gh = h @ w_hh.T + b_hh
        i_r, i_z, i_n = np.split(gi, 3, axis=-1)
        h_r, h_z, h_n = np.split(gh, 3, axis=-1)
        r = 1.0 / (1.0 + np.exp(-(i_r + h_r)))
        z = 1.0 / (1.0 + np.exp(-(i_z + h_z)))
        n = np.tanh(i_n + r * h_n)
        h = (1.0 - z) * n + z * h
        acc += h
    return acc / ARITY, h


def kernel(leaf_tokens, embed_table, w_ih, w_hh, b_ih, b_hh):
    from concourse.bass_utils import run_bass_kernel_spmd

    leaf_tokens = np.asarray(leaf_tokens, np.int32)
    embed_table = np.asarray(embed_table, np.float32)
    w_ih = np.asarray(w_ih, np.float32)
    w_hh = np.asarray(w_hh, np.float32)
    b_ih = np.asarray(b_ih, np.float32)
    b_hh = np.asarray(b_hh, np.float32)

    nc = _build_program()

    embed_bf = embed_table.astype(BF16)
    wpack = np.ascontiguousarray(
        np.stack(
            [
                _retile_weights(w_ih),
                _retile_weights(w_hh),
                _retile_weights(w_ih / ARITY),
            ],
            axis=1,
        )
    )
    biases = _prep_bias(b_ih, b_hh)
    bpack = np.ascontiguousarray(
        np.concatenate(
            [
                _prep_bias_mm(b_ih, b_hh).reshape(3, 4 * P),
                _prep_onehot3().reshape(3, 3 * 512),
            ],
            axis=1,
        )
    )
    in_maps = []
    for core in range(NCORES):
        in_maps.append(
            {
                "tokens32": _prep_tokens32(
                    leaf_tokens[core * LEAVES_CORE : (core + 1) * LEAVES_CORE]
                ),
                "embed": embed_bf,
                "wpack": wpack,
                "biases": biases,
                "bpack": bpack,
                "bpack6": _prep_bpack6(b_ih, b_hh),
            }
        )
    res = run_bass_kernel_spmd(nc, in_maps, core_ids=list(range(NCORES)))

    # device tensors -> [core, 64 nodes, 384] with f = j*128 + p
    x3 = np.zeros((NCORES, P4, DIM), np.float64)
    h3 = np.zeros((NCORES, P4, DIM), np.float64)
    for core in range(NCORES):
        hacc = np.asarray(res.results[core]["out_hacc"], np.float64)  # [128,3,64]
        hf = np.asarray(res.results[core]["out_hf"], np.float64)
        x3[core] = (hacc / ARITY).transpose(1, 0, 2).reshape(DIM, P4).T
        h3[core] = hf.transpose(1, 0, 2).reshape(DIM, P4).T

    w_ih64 = w_ih.astype(np.float64)
    w_hh64 = w_hh.astype(np.float64)
    b_ih64 = b_ih.astype(np.float64)
    b_hh64 = b_hh.astype(np.float64)

    # level 3: per core, 8 parents x 8 children (batch over cores*parents)
    xc = x3.reshape(NCORES * ARITY, ARITY, DIM).transpose(1, 0, 2)  # [A, 64, D]
    h0 = h3.reshape(NCORES * ARITY, ARITY, DIM).mean(axis=1)
    x2, h2 = _gru_level(xc, h0, w_ih64, w_hh64, b_ih64, b_hh64)

    # level 2: per core, 1 parent x 8 children
    xc = x2.reshape(NCORES, ARITY, DIM).transpose(1, 0, 2)  # [A, 8, D]
    h0 = h2.reshape(NCORES, ARITY, DIM).mean(axis=1)
    x1, h1 = _gru_level(xc, h0, w_ih64, w_hh64, b_ih64, b_hh64)

    # root: 8 cores' outputs
    xc = x1.reshape(1, ARITY, DIM).transpose(1, 0, 2)  # [A, 1, D]
    h0 = h1.reshape(1, ARITY, DIM).mean(axis=1)
    out, _ = _gru_level(xc, h0, w_ih64, w_hh64, b_ih64, b_hh64)

    return out.astype(np.float32).reshape(1, 1, DIM)


# revision 60
# speedup vs baseline: 1.2177x; 1.2177x over previous
"""Tree-GRU (arity-8, depth-5) over embedded leaves on 8 TRN2 NeuronCores.

Sharding: data-parallel over subtrees. Each core takes 4096 contiguous leaves
and runs levels 5 and 4 of the tree locally (512 -> 64 parents). The last two
per-core levels (64 -> 8 -> 1) and the root are small latency-bound GRU
cascades (free dim <= 8) done on host in fp64 after gathering the per-core
level-4 outputs, extending the baseline's host-side root reduction.

Device layout is feature-transposed: tensors live as [128 part, 3 ktile, ...]
with feature f = 128*k + p, so the GRU matmuls contract the partition dim.

Embeddings arrive per GRU step: tokens are host-permuted child-major, each
child's 512 rows fetched by 4 indirect DMAs (leaf-major) and flipped
feature-major by 4 xbar transpose-DMAs on the HWDGE rings — no tensor-engine
transposes, no PSUM, and the first GRU matmul can start after ~2 gathers.

Level 512 keeps one PSUM bank per (role, jo) output tile at N=512; each step
emits gi matmuls of units j0/j1 ahead of any hh matmul so the tensor engine
holds ~3.8us of h-independent work to hide the previous step's gate chain.
Unit j1 owns 4 banks (double-buffered step to step); j0 and j2 share the
other 4, with j2's allocation waiting on j0's progressively-freed banks
behind hh j1. Biases ride the scalar-activation bias port. The per-step
output accumulator is kept child-major (vector engine) so the final step
writes level 4's input directly as a fused raw-sum add; the 1/8 output-mean
scale is folded into a pre-scaled copy of W_ih used by level 4. Level 64
injects biases into PSUM via a K=3 one-hot matmul (the only start=True
write), collapsing the gate chain to jo-spanning instructions.
"""

import numpy as np
import ml_dtypes

ARITY = 8
DIM = 384
VOCAB = 32000
NCORES = 8
P = 128
J = 3  # DIM // 128 feature tiles
N_LEAVES = 32768
LEAVES_CORE = N_LEAVES // NCORES  # 4096
P5 = LEAVES_CORE // ARITY  # 512 level-5 parents per core
P4 = P5 // ARITY  # 64 level-4 parents per core
GT = P5 // P  # 4 gather tiles per child

BF16 = ml_dtypes.bfloat16

_PROG_CACHE = {}


def _emit(tc, nc, aps):
    import concourse.mybir as mybir
    import concourse.bass as bass
    from concourse.masks import make_identity

    f32 = mybir.dt.float32
    bf16 = mybir.dt.bfloat16
    Sig = mybir.ActivationFunctionType.Sigmoid
    Tanh = mybir.ActivationFunctionType.Tanh
    Add = mybir.AluOpType.add
    Sub = mybir.AluOpType.subtract
    Mult = mybir.AluOpType.mult

    tokens32, embed, wih_t, biases, biases_mm, bpack6, out_hacc, out_hf = aps

    from contextlib import ExitStack

    with ExitStack() as ctx:
        const = ctx.enter_context(tc.tile_pool(name="const", bufs=1))
        xpool = ctx.enter_context(tc.tile_pool(name="xpool", bufs=1))
        gpool = ctx.enter_context(tc.tile_pool(name="gpool", bufs=3))
        state = ctx.enter_context(tc.tile_pool(name="state", bufs=1))
        gates = ctx.enter_context(tc.tile_pool(name="gates", bufs=4))
        pspool = ctx.enter_context(tc.tile_pool(name="pspool", bufs=4, space="PSUM"))
        pspool2 = ctx.enter_context(tc.tile_pool(name="pspool2", bufs=4, space="PSUM"))

        # ---- token tile first, then per-child embedding gathers ----
        # all children are fetched leaf-major by native indirect DMAs (no
        # GpSimd library, starts right after the token DMA) and flipped
        # feature-major by tensor-engine transposes two steps ahead of use.
        tok32_sb = const.tile([P, ARITY * GT], mybir.dt.int32)
        nc.sync.dma_start(tok32_sb[:], tokens32[:])

        wpack_sb = const.tile([P, 3, J, 9, P], bf16)
        wih_sb = wpack_sb[:, 0]
        whh_sb = wpack_sb[:, 1]
        wih_s_sb = wpack_sb[:, 2]
        bias_sb = const.tile([P, 12], f32)
        bpack_sb = const.tile([3, 4 * P + 3 * 512], bf16)
        bias3_sb = bpack_sb[:, : 4 * P].rearrange("k (r p) -> k r p", r=4)
        onehot3_sb = bpack_sb[:, 4 * P :].rearrange("k (j n) -> k j n", j=3)
        bpack6_sb = const.tile([6, P + 6 * P4], bf16)
        bias6_sb = bpack6_sb[:, :P]
        onehot6_sb = bpack6_sb[:, P:].rearrange("k (g n) -> k g n", g=6)
        nc.sync.dma_start(wpack_sb[:], wih_t[:])
        nc.sync.dma_start(bias_sb[:], biases[:])
        nc.sync.dma_start(bpack_sb[:], biases_mm[:])
        nc.sync.dma_start(bpack6_sb[:], bpack6[:])

        # x5[p, child, j, q]
        x5 = xpool.tile([P, ARITY, J, P5], bf16, name="x5", tag="x5")
        ident = const.tile([P, P], bf16)
        make_identity(nc, ident[:])

        xgs = {}
        for t in range(ARITY):
            c = ARITY - 1 - t  # children consumed in reverse: child 7 first
            xg = gpool.tile([P, GT, DIM], bf16, name="xg", tag="xg")
            xgs[c] = xg
            for g in range(GT):
                gi_inst = nc.gpsimd.indirect_dma_start(
                    out=xg[:, g, :],
                    out_offset=None,
                    in_=embed[:],
                    in_offset=bass.IndirectOffsetOnAxis(
                        ap=tok32_sb[:, c * GT + g : c * GT + g + 1], axis=0
                    ),
                )
                if (t * GT + g) % 2 == 1:
                    gi_inst.ins.queue = "qPoolDynamic1"

        def emit_transposes(c):
            # 4 gather tiles of one feature third -> one PSUM bank, one copy
            xg = xgs[c]
            for j in range(J):
                tp = pspool2.tile([P, 512], bf16, name="tp", tag="ps2")
                for g in range(GT):
                    nc.tensor.transpose(
                        tp[:, g * P : (g + 1) * P],
                        xg[:, g, j * P : (j + 1) * P],
                        ident[:],
                    )
                nc.vector.tensor_copy(out=x5[:, c, j, :], in_=tp[:])

        emit_transposes(7)
        emit_transposes(6)

        x4 = xpool.tile([P, ARITY, J, P4], bf16, name="x4", tag="x4")

        def psum_tile(jo):
            # 8 banks for 12 role-tiles per step: unit j1 owns pspool (reuse
            # waits on the previous step's j1 gates); j0/j2 share pspool2 —
            # j2 waits on same-step j0 gates (freed progressively under hh
            # j1), j0 on the previous step's j2 gates. All waits point at
            # strictly earlier FIFO positions: no deadlock.
            if jo == 1:
                return pspool.tile([P, 512], f32, name="ps", tag="ps")
            return pspool2.tile([P, 512], f32, name="ps2", tag="ps2")

        # =================== level 5: 512 parents, leaf children ===================
        h5 = state.tile([P, J, P5], bf16, name="h5", tag="h5")
        hacc5 = state.tile([P, J, P5], f32, name="hacc5", tag="hacc5")
        nc.gpsimd.memset(hacc5[:], 0.0)
        csum5 = state.tile([P, J, P4], f32, name="csum5", tag="csum5")

        with nc.named_scope("level_512"):
            for t in range(ARITY):
                c = ARITY - 1 - t
                leaf0 = t == 0

                ps_r = [None] * J
                ps_z = [None] * J
                ps_in = [None] * J
                ps_hn = [None] * J

                def emit_gi(jo):
                    ps_r[jo] = psum_tile(jo)
                    ps_z[jo] = psum_tile(jo)
                    ps_in[jo] = psum_tile(jo)
                    if not leaf0:
                        ps_hn[jo] = psum_tile(jo)
                    for ps, moff in ((ps_r[jo], 0), (ps_z[jo], 3), (ps_in[jo], 6)):
                        for ji in range(J):
                            nc.tensor.matmul(
                                ps[:, :P5],
                                wih_sb[:, ji, moff + jo, :],
                                x5[:, c, ji, :],
                                start=(ji == 0),
                                stop=(ji == 2 and (moff == 6 or leaf0)),
                            )

                def emit_hh(jo):
                    if leaf0:
                        return
                    for ps, moff in ((ps_r[jo], 0), (ps_z[jo], 3), (ps_hn[jo], 6)):
                        for ji in range(J):
                            nc.tensor.matmul(
                                ps[:, :P5],
                                whh_sb[:, ji, moff + jo, :],
                                h5[:, ji, :],
                                start=(ji == 0 and moff == 6),
                                stop=(ji == 2),
                            )

                emit_gi(0)
                emit_gi(1)
                if t <= 5:
                    # next-next child's feature flip rides behind ~3.8us of
                    # gi matmuls so its PSUM-bank wait never stalls the PE
                    emit_transposes(5 - t)
                emit_hh(0)
                emit_hh(1)
                emit_gi(2)
                emit_hh(2)

                for jo in range(J):
                    r_sb = gates.tile([P, P5], bf16, name="r_sb", tag="r_sb")
                    z_sb = gates.tile([P, P5], bf16, name="z_sb", tag="z_sb")
                    n_sb = gates.tile([P, P5], bf16, name="n_sb", tag="n_sb")
                    rhn = gates.tile([P, P5], f32, name="rhn", tag="rhn")
                    t1 = gates.tile([P, P5], bf16, name="t1", tag="t1")

                    nc.scalar.activation(
                        r_sb[:], ps_r[jo][:, :P5], Sig, bias=bias_sb[:, jo : jo + 1]
                    )
                    nc.scalar.activation(
                        z_sb[:], ps_z[jo][:, :P5], Sig, bias=bias_sb[:, 3 + jo : 4 + jo]
                    )
                    if leaf0:
                        nc.vector.tensor_scalar_mul(
                            rhn[:], r_sb[:], bias_sb[:, 6 + jo : 7 + jo]
                        )
                    else:
                        nc.vector.scalar_tensor_tensor(
                            out=rhn[:],
                            in0=ps_hn[jo][:, :P5],
                            scalar=bias_sb[:, 6 + jo : 7 + jo],
                            in1=r_sb[:],
                            op0=Add,
                            op1=Mult,
                        )
                    nc.vector.tensor_tensor(
                        out=rhn[:], in0=rhn[:], in1=ps_in[jo][:, :P5], op=Add
                    )
                    nc.scalar.activation(
                        n_sb[:], rhn[:], Tanh, bias=bias_sb[:, 9 + jo : 10 + jo]
                    )
                    hsl = h5[:, jo, :]
                    if leaf0:
                        nc.vector.tensor_tensor(out=t1[:], in0=z_sb[:], in1=n_sb[:], op=Mult)
                        nc.vector.tensor_tensor(out=hsl, in0=n_sb[:], in1=t1[:], op=Sub)
                    else:
                        nc.vector.tensor_tensor(out=t1[:], in0=hsl, in1=n_sb[:], op=Sub)
                        nc.vector.tensor_tensor(out=t1[:], in0=z_sb[:], in1=t1[:], op=Mult)
                        nc.vector.tensor_tensor(out=hsl, in0=n_sb[:], in1=t1[:], op=Add)

                # output accumulation after the chain ops so it never delays
                # the next step's recurrent matmuls
                for jo in range(J):
                    hsl = h5[:, jo, :]
                    if t == ARITY - 1:
                        nc.vector.tensor_reduce(
                            out=csum5[:, jo, :],
                            in_=hsl.rearrange("p (q c) -> p q c", c=ARITY),
                            axis=mybir.AxisListType.X,
                            op=Add,
                        )
                        # the only permuted pass: per-child writes, child 7
                        # first, so level 64's first gi matmuls unblock early
                        haccp = hacc5[:, jo].rearrange("p (q c) -> p c q", c=ARITY)
                        hperm = hsl.rearrange("p (q c) -> p c q", c=ARITY)
                        for cc in range(ARITY - 1, -1, -1):
                            nc.vector.tensor_tensor(
                                out=x4[:, cc, jo, :],
                                in0=haccp[:, cc],
                                in1=hperm[:, cc],
                                op=Add,
                            )
                    else:
                        # contiguous accumulate: strided reads here would
                        # triple the op cost and starve the gate chain
                        nc.vector.tensor_tensor(
                            out=hacc5[:, jo], in0=hacc5[:, jo], in1=hsl, op=Add
                        )

        # =================== level 4: 64 parents ===================
        h4 = state.tile([P, J, P4], bf16, name="h4", tag="h4")
        nc.scalar.mul(h4[:], csum5[:], 1.0 / ARITY)
        hacc4 = state.tile([P, J, P4], f32, name="hacc4", tag="hacc4")
        nc.gpsimd.memset(hacc4[:], 0.0)
        N3 = J * P4  # 192

        with nc.named_scope("level_64"):
            for t in range(ARITY):
                c = ARITY - 1 - t
                pool = pspool if t % 2 == 0 else pspool2
                tag = "ps" if t % 2 == 0 else "ps2"
                ps_rz, ps_hn, ps_in = (
                    pool.tile([P, 512], f32, name=tag, tag=tag) for _ in range(3)
                )

                def view3(pst):
                    return pst[:, :N3].rearrange("p (j n) -> p j n", j=3)

                # bias matmuls: the only start=True writes. r and z share one
                # tile via a K=6 one-hot so a single sigmoid covers both.
                nc.tensor.matmul(
                    ps_rz[:, : 2 * N3],
                    bias6_sb[:],
                    onehot6_sb[:],
                    start=True,
                    stop=False,
                )
                for pst, ro in ((ps_hn, 2), (ps_in, 3)):
                    nc.tensor.matmul(
                        pst[:, :N3],
                        bias3_sb[:, ro, :],
                        onehot3_sb[:, :, :P4],
                        start=True,
                        stop=False,
                    )
                for off, moff in ((0, 0), (N3, 3), (None, 6)):
                    pst, base = (ps_in, 0) if off is None else (ps_rz, off)
                    for jo in range(J):
                        for ji in range(J):
                            nc.tensor.matmul(
                                pst[:, base + jo * P4 : base + (jo + 1) * P4],
                                wih_s_sb[:, ji, moff + jo, :],
                                x4[:, c, ji, :],
                                start=False,
                                stop=(moff == 6 and jo == 2 and ji == 2),
                            )
                for off, moff in ((0, 0), (N3, 3), (None, 6)):
                    pst, base = (ps_hn, 0) if off is None else (ps_rz, off)
                    for jo in range(J):
                        for ji in range(J):
                            nc.tensor.matmul(
                                pst[:, base + jo * P4 : base + (jo + 1) * P4],
                                whh_sb[:, ji, moff + jo, :],
                                h4[:, ji, :],
                                start=False,
                                stop=(jo == 2 and ji == 2 and moff != 0),
                            )

                rz_sb = gates.tile([P, 2, J, P4], bf16, name="rz4", tag="rz4")
                n_sb = gates.tile([P, J, P4], bf16, name="n4", tag="n4")
                rhn = gates.tile([P, J, P4], f32, name="rhn4", tag="rhn4")
                t1 = gates.tile([P, J, P4], bf16, name="t14", tag="t14")
                r_sb = rz_sb[:, 0]
                z_sb = rz_sb[:, 1]

                nc.scalar.activation(
                    rz_sb[:],
                    ps_rz[:, : 2 * N3].rearrange("p (r j n) -> p r j n", r=2, j=3),
                    Sig,
                )
                nc.vector.tensor_tensor(
                    out=rhn[:], in0=view3(ps_hn), in1=r_sb, op=Mult
                )
                nc.vector.tensor_tensor(
                    out=rhn[:], in0=rhn[:], in1=view3(ps_in), op=Add
                )
                nc.scalar.activation(n_sb[:], rhn[:], Tanh)
                nc.vector.tensor_tensor(out=t1[:], in0=h4[:], in1=n_sb[:], op=Sub)
                nc.vector.tensor_tensor(out=t1[:], in0=z_sb, in1=t1[:], op=Mult)
                nc.vector.tensor_tensor(out=h4[:], in0=n_sb[:], in1=t1[:], op=Add)
                # off the critical chain: gpsimd is idle during level 64
                nc.gpsimd.tensor_tensor(
                    out=hacc4[:], in0=hacc4[:], in1=h4[:], op=Add
                )

        # ---- outputs: raw h-sum (x3*8) and final hiddens of the 64 nodes ----
        nc.sync.dma_start(out_hacc[:], hacc4[:])
        nc.sync.dma_start(out_hf[:], h4[:])


def _build_program():
    if "prog" in _PROG_CACHE:
        return _PROG_CACHE["prog"]
    import concourse.bacc as bacc
    import concourse.mybir as mybir
    import concourse.tile as tile

    f32 = mybir.dt.float32
    bf16 = mybir.dt.bfloat16

    nc = bacc.Bacc(
        "TRN2",
        target_bir_lowering=False,
        debug=False,
        enable_asserts=False,
        num_devices=NCORES,
        num_swdge_queues=2,
    )
    tokens32 = nc.dram_tensor(
        "tokens32", [P, ARITY * GT], mybir.dt.int32, kind="ExternalInput"
    ).ap()
    embed = nc.dram_tensor("embed", [VOCAB, DIM], bf16, kind="ExternalInput").ap()
    wpack = nc.dram_tensor("wpack", [P, 3, J, 9, P], bf16, kind="ExternalInput").ap()
    biases = nc.dram_tensor("biases", [P, 12], f32, kind="ExternalInput").ap()
    bpack = nc.dram_tensor(
        "bpack", [3, 4 * P + 3 * 512], bf16, kind="ExternalInput"
    ).ap()
    bpack6 = nc.dram_tensor(
        "bpack6", [6, P + 6 * P4], bf16, kind="ExternalInput"
    ).ap()
    out_hacc = nc.dram_tensor("out_hacc", [P, J, P4], f32, kind="ExternalOutput").ap()
    out_hf = nc.dram_tensor("out_hf", [P, J, P4], bf16, kind="ExternalOutput").ap()

    with tile.TileContext(nc) as tc:
        _emit(tc, nc, (tokens32, embed, wpack, biases, bpack, bpack6, out_hacc, out_hf))
    nc.compile()
    _PROG_CACHE["prog"] = nc
    return nc


def _retile_weights(w):
    # w: [1152, 384] -> lhsT tiles [128(k_part), 3(k), 9(m), 128(m_col)] bf16
    wt = np.ascontiguousarray(w.T)  # [384, 1152]
    wt = wt.reshape(J, P, 9, P).transpose(1, 0, 2, 3)
    return np.ascontiguousarray(wt).astype(BF16)


def _prep_bias(b_ih, b_hh):
    biases = np.zeros((P, 12), np.float32)
    comb = (b_ih + b_hh).reshape(9, P)
    biases[:, 0:6] = comb[0:6].T
    biases[:, 6:9] = b_hh.reshape(9, P)[6:9].T
    biases[:, 9:12] = b_ih.reshape(9, P)[6:9].T
    return biases


def _prep_bias_mm(b_ih, b_hh):
    # lhsT[k, ro, q] = bias[q, 3*ro + k]: the K=3 bias matmul against the
    # one-hot rhs yields out[q, (j, n)] = bias[q, 3*ro + j].
    b = _prep_bias(b_ih, b_hh)  # [128, 12] cols: r0..2 z0..2 hn0..2 in0..2
    out = b.T.reshape(4, 3, P).transpose(1, 0, 2)
    return np.ascontiguousarray(out).astype(BF16)


def _prep_onehot3():
    out = np.zeros((3, 3, 512), np.float32)
    for k in range(3):
        out[k, k, :] = 1.0
    return out.astype(BF16)


def _prep_bpack6(b_ih, b_hh):
    # K=6 bias matmul for the merged r+z PSUM tile: lhsT rows are the six
    # combined bias vectors (r jo0..2, z jo0..2), rhs is a [6, 6, 64] one-hot.
    b = _prep_bias(b_ih, b_hh)  # [128, 12]
    lhs = b[:, 0:6].T.astype(np.float32)  # [6, 128]
    oh = np.zeros((6, 6, P4), np.float32)
    for k in range(6):
        oh[k, k, :] = 1.0
    out = np.concatenate([lhs, oh.reshape(6, 6 * P4)], axis=1)
    return np.ascontiguousarray(out).astype(BF16)


def _prep_tokens32(tokens_core):
    # int32 indirect-DMA tokens: col c*4+g, row p holds tokens[(g*128+p)*8+c]
    tok = tokens_core.reshape(P5, ARITY).T  # [8 child, 512 parent]
    sel = tok.reshape(ARITY, GT, P).transpose(2, 0, 1).reshape(P, ARITY * GT)
    return np.ascontiguousarray(sel)


def _gru_level(x_children, h0, w_ih, w_hh, b_ih, b_hh):
    # x_children: [A, N, D] in original child order; consumed reversed.
    h = h0
    acc = np.zeros_like(h)
    for t in range(ARITY):
        x_t = x_children[ARITY - 1 - t]
        gi = x_t @ w_ih.T + b_ih
        gh = h @ w_hh.T + b_hh
        i_r, i_z, i_n = np.split(gi, 3, axis=-1)
        h_r, h_z, h_n = np.split(gh, 3, axis=-1)
        r = 1.0 / (1.0 + np.exp(-(i_r + h_r)))
        z = 1.0 / (1.0 + np.exp(-(i_z + h_z)))
        n = np.tanh(i_n + r * h_n)
        h = (1.0 - z) * n + z * h
        acc += h
    return acc / ARITY, h


def kernel(leaf_tokens, embed_table, w_ih, w_hh, b_ih, b_hh):
    from concourse.bass_utils import run_bass_kernel_spmd

    leaf_tokens = np.asarray(leaf_tokens, np.int32)
    embed_table = np.asarray(embed_table, np.float32)
    w_ih = np.asarray(w_ih, np.float32)
    w_hh = np.asarray(w_hh, np.float32)
    b_ih = np.asarray(b_ih, np.float32)
    b_hh = np.asarray(b_hh, np.float32)

    nc = _build_program()

    embed_bf = embed_table.astype(BF16)
    wpack = np.ascontiguousarray(
        np.stack(
            [
                _retile_weights(w_ih),
                _retile_weights(w_hh),
                _retile_weights(w_ih / ARITY),
            ],
            axis=1,
        )
    )
    biases = _prep_bias(b_ih, b_hh)
    bpack = np.ascontiguousarray(
        np.concatenate(
            [
                _prep_bias_mm(b_ih, b_hh).reshape(3, 4 * P),
                _prep_onehot3().reshape(3, 3 * 512),
            ],
            axis=1,
        )
    )
    in_maps = []
    for core in range(NCORES):
        in_maps.append(
            {
                "tokens32": _prep_tokens32(
                    leaf_tokens[core * LEAVES_CORE : (core + 1) * LEAVES_CORE]
                ),
                "embed": embed_bf,
                "wpack": wpack,
                "biases": biases,
                "bpack": bpack,
                "bpack6": _prep_bpack6(b_ih, b_hh),
            }
        )
    res = run_bass_kernel_spmd(nc, in_maps, core_ids=list(range(NCORES)))

    # device tensors -> [core, 64 nodes, 384] with f = j*128 + p
    x3 = np.zeros((NCORES, P4, DIM), np.float64)
    h3 = np.zeros((NCORES, P4, DIM), np.float64)
    for core in range(NCORES):
        hacc = np.asarray(res.results[core]["out_hacc"], np.float64)  # [128,3,64]
        hf = np.asarray(res.results[core]["out_hf"], np.float64)
        x3[core] = (hacc / ARITY).transpose(1, 0, 2).reshape(DIM, P4).T
        h3[core] = hf.transpose(1, 0, 2).reshape(DIM, P4).T

    w_ih64 = w_ih.astype(np.float64)
    w_hh64 = w_hh.astype(np.float64)
    b_ih64 = b_ih.astype(np.float64)
    b_hh64 = b_hh.astype(np.float64)

    # level 3: per core, 8 parents x 8 children (batch over cores*parents)
    xc = x3.reshape(NCORES * ARITY, ARITY, DIM).transpose(1, 0, 2)  # [A, 64, D]
    h0 = h3.reshape(NCORES * ARITY, ARITY, DIM).mean(axis=1)
    x2, h2 = _gru_level(xc, h0, w_ih64, w_hh64, b_ih64, b_hh64)

    # level 2: per core, 1 parent x 8 children
    xc = x2.reshape(NCORES, ARITY, DIM).transpose(1, 0, 2)  # [A, 8, D]
    h0 = h2.reshape(NCORES, ARITY, DIM).mean(axis=1)
    x1, h1 = _gru_level(xc, h0, w_ih64, w_hh64, b_ih64, b_hh64)

    # root: 8 cores' outputs
    xc = x1.reshape(1, ARITY, DIM).transpose(1, 0, 2)  # [A, 1, D]
    h0 = h1.reshape(1, ARITY, DIM).mean(axis=1)
    out, _ = _gru_level(xc, h0, w_ih64, w_hh64, b_ih64, b_hh64)

    return out.astype(np.float32).reshape(1, 1, DIM)


# revision 61
# speedup vs baseline: 1.2245x; 1.0056x over previous
"""Tree-GRU (arity-8, depth-5) over embedded leaves on 8 TRN2 NeuronCores.

Sharding: data-parallel over subtrees. Each core takes 4096 contiguous leaves
and runs levels 5 and 4 of the tree locally (512 -> 64 parents). The last two
per-core levels (64 -> 8 -> 1) and the root are small latency-bound GRU
cascades (free dim <= 8) done on host in fp64 after gathering the per-core
level-4 outputs, extending the baseline's host-side root reduction.

Device layout is feature-transposed: tensors live as [128 part, 3 ktile, ...]
with feature f = 128*k + p, so the GRU matmuls contract the partition dim.

Embeddings arrive per GRU step: tokens are host-permuted child-major, each
child's 512 rows fetched by 4 indirect DMAs (leaf-major) and flipped
feature-major by 4 xbar transpose-DMAs on the HWDGE rings — no tensor-engine
transposes, no PSUM, and the first GRU matmul can start after ~2 gathers.

Level 512 keeps one PSUM bank per (role, jo) output tile at N=512; each step
emits gi matmuls of units j0/j1 ahead of any hh matmul so the tensor engine
holds ~3.8us of h-independent work to hide the previous step's gate chain.
Unit j1 owns 4 banks (double-buffered step to step); j0 and j2 share the
other 4, with j2's allocation waiting on j0's progressively-freed banks
behind hh j1. Biases ride the scalar-activation bias port. The per-step
output accumulator is kept child-major (vector engine) so the final step
writes level 4's input directly as a fused raw-sum add; the 1/8 output-mean
scale is folded into a pre-scaled copy of W_ih used by level 4. Level 64
injects biases into PSUM via a K=3 one-hot matmul (the only start=True
write), collapsing the gate chain to jo-spanning instructions.
"""

import numpy as np
import ml_dtypes

ARITY = 8
DIM = 384
VOCAB = 32000
NCORES = 8
P = 128
J = 3  # DIM // 128 feature tiles
N_LEAVES = 32768
LEAVES_CORE = N_LEAVES // NCORES  # 4096
P5 = LEAVES_CORE // ARITY  # 512 level-5 parents per core
P4 = P5 // ARITY  # 64 level-4 parents per core
GT = P5 // P  # 4 gather tiles per child

BF16 = ml_dtypes.bfloat16

_PROG_CACHE = {}


def _emit(tc, nc, aps):
    import concourse.mybir as mybir
    import concourse.bass as bass
    from concourse.masks import make_identity

    f32 = mybir.dt.float32
    bf16 = mybir.dt.bfloat16
    Sig = mybir.ActivationFunctionType.Sigmoid
    Tanh = mybir.ActivationFunctionType.Tanh
    Add = mybir.AluOpType.add
    Sub = mybir.AluOpType.subtract
    Mult = mybir.AluOpType.mult

    tokens32, embed, wih_t, biases, biases_mm, bpack6, out_hacc, out_hf = aps

    from contextlib import ExitStack

    with ExitStack() as ctx:
        const = ctx.enter_context(tc.tile_pool(name="const", bufs=1))
        xpool = ctx.enter_context(tc.tile_pool(name="xpool", bufs=1))
        gpool = ctx.enter_context(tc.tile_pool(name="gpool", bufs=3))
        state = ctx.enter_context(tc.tile_pool(name="state", bufs=1))
        gates = ctx.enter_context(tc.tile_pool(name="gates", bufs=4))
        pspool = ctx.enter_context(tc.tile_pool(name="pspool", bufs=4, space="PSUM"))
        pspool2 = ctx.enter_context(tc.tile_pool(name="pspool2", bufs=4, space="PSUM"))

        # ---- token tile first, then per-child embedding gathers ----
        # all children are fetched leaf-major by native indirect DMAs (no
        # GpSimd library, starts right after the token DMA) and flipped
        # feature-major by tensor-engine transposes two steps ahead of use.
        tok32_sb = const.tile([P, ARITY * GT], mybir.dt.int32)
        nc.sync.dma_start(tok32_sb[:], tokens32[:])

        wpack_sb = const.tile([P, 3, J, 9, P], bf16)
        wih_sb = wpack_sb[:, 0]
        whh_sb = wpack_sb[:, 1]
        wih_s_sb = wpack_sb[:, 2]
        bias_sb = const.tile([P, 12], f32)
        bpack_sb = const.tile([3, 4 * P + 3 * 512], bf16)
        bias3_sb = bpack_sb[:, : 4 * P].rearrange("k (r p) -> k r p", r=4)
        onehot3_sb = bpack_sb[:, 4 * P :].rearrange("k (j n) -> k j n", j=3)
        bpack6_sb = const.tile([6, P + 6 * P4], bf16)
        bias6_sb = bpack6_sb[:, :P]
        onehot6_sb = bpack6_sb[:, P:].rearrange("k (g n) -> k g n", g=6)
        nc.sync.dma_start(wpack_sb[:], wih_t[:])
        nc.sync.dma_start(bias_sb[:], biases[:])
        nc.sync.dma_start(bpack_sb[:], biases_mm[:])
        nc.sync.dma_start(bpack6_sb[:], bpack6[:])

        # x5[p, child, j, q]
        x5 = xpool.tile([P, ARITY, J, P5], bf16, name="x5", tag="x5")
        ident = const.tile([P, P], bf16)
        make_identity(nc, ident[:])

        xgs = {}
        for t in range(ARITY):
            c = ARITY - 1 - t  # children consumed in reverse: child 7 first
            xg = gpool.tile([P, GT, DIM], bf16, name="xg", tag="xg")
            xgs[c] = xg
            for g in range(GT):
                gi_inst = nc.gpsimd.indirect_dma_start(
                    out=xg[:, g, :],
                    out_offset=None,
                    in_=embed[:],
                    in_offset=bass.IndirectOffsetOnAxis(
                        ap=tok32_sb[:, c * GT + g : c * GT + g + 1], axis=0
                    ),
                )
                if (t * GT + g) % 2 == 1:
                    gi_inst.ins.queue = "qPoolDynamic1"

        def emit_transposes(c):
            # 4 gather tiles of one feature third -> one PSUM bank, one copy
            xg = xgs[c]
            for j in range(J):
                tp = pspool2.tile([P, 512], bf16, name="tp", tag="ps2")
                for g in range(GT):
                    nc.tensor.transpose(
                        tp[:, g * P : (g + 1) * P],
                        xg[:, g, j * P : (j + 1) * P],
                        ident[:],
                    )
                nc.vector.tensor_copy(out=x5[:, c, j, :], in_=tp[:])

        emit_transposes(7)
        emit_transposes(6)

        x4 = xpool.tile([P, ARITY, J, P4], bf16, name="x4", tag="x4")

        def psum_tile(jo):
            # 8 banks for 12 role-tiles per step: unit j1 owns pspool (reuse
            # waits on the previous step's j1 gates); j0/j2 share pspool2 —
            # j2 waits on same-step j0 gates (freed progressively under hh
            # j1), j0 on the previous step's j2 gates. All waits point at
            # strictly earlier FIFO positions: no deadlock.
            if jo == 1:
                return pspool.tile([P, 512], f32, name="ps", tag="ps")
            return pspool2.tile([P, 512], f32, name="ps2", tag="ps2")

        # =================== level 5: 512 parents, leaf children ===================
        h5 = state.tile([P, J, P5], bf16, name="h5", tag="h5")
        hacc5 = state.tile([P, J, P5], f32, name="hacc5", tag="hacc5")
        nc.gpsimd.memset(hacc5[:], 0.0)
        csum5 = state.tile([P, J, P4], f32, name="csum5", tag="csum5")

        with nc.named_scope("level_512"):
            for t in range(ARITY):
                c = ARITY - 1 - t
                leaf0 = t == 0

                ps_r = [None] * J
                ps_z = [None] * J
                ps_in = [None] * J
                ps_hn = [None] * J

                def emit_gi(jo):
                    ps_r[jo] = psum_tile(jo)
                    ps_z[jo] = psum_tile(jo)
                    ps_in[jo] = psum_tile(jo)
                    if not leaf0:
                        ps_hn[jo] = psum_tile(jo)
                    for ps, moff in ((ps_r[jo], 0), (ps_z[jo], 3), (ps_in[jo], 6)):
                        for ji in range(J):
                            nc.tensor.matmul(
                                ps[:, :P5],
                                wih_sb[:, ji, moff + jo, :],
                                x5[:, c, ji, :],
                                start=(ji == 0),
                                stop=(ji == 2 and (moff == 6 or leaf0)),
                            )

                def emit_hh(jo):
                    if leaf0:
                        return
                    for ps, moff in ((ps_r[jo], 0), (ps_z[jo], 3), (ps_hn[jo], 6)):
                        for ji in range(J):
                            nc.tensor.matmul(
                                ps[:, :P5],
                                whh_sb[:, ji, moff + jo, :],
                                h5[:, ji, :],
                                start=(ji == 0 and moff == 6),
                                stop=(ji == 2),
                            )

                emit_gi(0)
                emit_gi(1)
                if t <= 5:
                    # next-next child's feature flip rides behind ~3.8us of
                    # gi matmuls so its PSUM-bank wait never stalls the PE
                    emit_transposes(5 - t)
                emit_hh(0)
                emit_hh(1)
                emit_gi(2)
                emit_hh(2)

                for jo in range(J):
                    r_sb = gates.tile([P, P5], bf16, name="r_sb", tag="r_sb")
                    z_sb = gates.tile([P, P5], bf16, name="z_sb", tag="z_sb")
                    n_sb = gates.tile([P, P5], bf16, name="n_sb", tag="n_sb")
                    rhn = gates.tile([P, P5], f32, name="rhn", tag="rhn")
                    t1 = gates.tile([P, P5], bf16, name="t1", tag="t1")

                    nc.scalar.activation(
                        r_sb[:], ps_r[jo][:, :P5], Sig, bias=bias_sb[:, jo : jo + 1]
                    )
                    nc.scalar.activation(
                        z_sb[:], ps_z[jo][:, :P5], Sig, bias=bias_sb[:, 3 + jo : 4 + jo]
                    )
                    if leaf0:
                        nc.vector.tensor_scalar_mul(
                            rhn[:], r_sb[:], bias_sb[:, 6 + jo : 7 + jo]
                        )
                    else:
                        nc.vector.scalar_tensor_tensor(
                            out=rhn[:],
                            in0=ps_hn[jo][:, :P5],
                            scalar=bias_sb[:, 6 + jo : 7 + jo],
                            in1=r_sb[:],
                            op0=Add,
                            op1=Mult,
                        )
                    nc.vector.tensor_tensor(
                        out=rhn[:], in0=rhn[:], in1=ps_in[jo][:, :P5], op=Add
                    )
                    nc.scalar.activation(
                        n_sb[:], rhn[:], Tanh, bias=bias_sb[:, 9 + jo : 10 + jo]
                    )
                    hsl = h5[:, jo, :]
                    if leaf0:
                        nc.vector.tensor_tensor(out=t1[:], in0=z_sb[:], in1=n_sb[:], op=Mult)
                        nc.vector.tensor_tensor(out=hsl, in0=n_sb[:], in1=t1[:], op=Sub)
                    else:
                        nc.vector.tensor_tensor(out=t1[:], in0=hsl, in1=n_sb[:], op=Sub)
                        nc.vector.tensor_tensor(out=t1[:], in0=z_sb[:], in1=t1[:], op=Mult)
                        nc.vector.tensor_tensor(out=hsl, in0=n_sb[:], in1=t1[:], op=Add)

                # output accumulation after the chain ops so it never delays
                # the next step's recurrent matmuls
                for jo in range(J):
                    hsl = h5[:, jo, :]
                    if t == ARITY - 1:
                        nc.vector.tensor_reduce(
                            out=csum5[:, jo, :],
                            in_=hsl.rearrange("p (q c) -> p q c", c=ARITY),
                            axis=mybir.AxisListType.X,
                            op=Add,
                        )
                        # the only permuted pass: per-child writes, child 7
                        # first, so level 64's first gi matmuls unblock early
                        haccp = hacc5[:, jo].rearrange("p (q c) -> p c q", c=ARITY)
                        hperm = hsl.rearrange("p (q c) -> p c q", c=ARITY)
                        for cc in range(ARITY - 1, -1, -1):
                            nc.vector.tensor_tensor(
                                out=x4[:, cc, jo, :],
                                in0=haccp[:, cc],
                                in1=hperm[:, cc],
                                op=Add,
                            )
                    else:
                        # contiguous accumulate: strided reads here would
                        # triple the op cost and starve the gate chain
                        nc.vector.tensor_tensor(
                            out=hacc5[:, jo], in0=hacc5[:, jo], in1=hsl, op=Add
                        )

        # =================== level 4: 64 parents ===================
        h4 = state.tile([P, J, P4], bf16, name="h4", tag="h4")
        nc.scalar.mul(h4[:], csum5[:], 1.0 / ARITY)
        hacc4 = state.tile([P, J, P4], f32, name="hacc4", tag="hacc4")
        nc.gpsimd.memset(hacc4[:], 0.0)
        N3 = J * P4  # 192

        def emit_bias_gi4(t):
            # bias matmuls are the only start=True writes; r and z share one
            # tile via a K=6 one-hot so a single sigmoid covers both. Emitted
            # one step ahead (x4 and the previous banks are ready) so these
            # ~30 matmuls keep the tensor engine busy under the gate chain.
            c = ARITY - 1 - t
            pool = pspool if t % 2 == 0 else pspool2
            tag = "ps" if t % 2 == 0 else "ps2"
            ps_rz, ps_hn, ps_in = (
                pool.tile([P, 512], f32, name=tag, tag=tag) for _ in range(3)
            )
            nc.tensor.matmul(
                ps_rz[:, : 2 * N3],
                bias6_sb[:],
                onehot6_sb[:],
                start=True,
                stop=False,
            )
            for pst, ro in ((ps_hn, 2), (ps_in, 3)):
                nc.tensor.matmul(
                    pst[:, :N3],
                    bias3_sb[:, ro, :],
                    onehot3_sb[:, :, :P4],
                    start=True,
                    stop=False,
                )
            for off, moff in ((0, 0), (N3, 3), (None, 6)):
                pst, base = (ps_in, 0) if off is None else (ps_rz, off)
                for jo in range(J):
                    for ji in range(J):
                        nc.tensor.matmul(
                            pst[:, base + jo * P4 : base + (jo + 1) * P4],
                            wih_s_sb[:, ji, moff + jo, :],
                            x4[:, c, ji, :],
                            start=False,
                            stop=(moff == 6 and jo == 2 and ji == 2),
                        )
            return ps_rz, ps_hn, ps_in

        with nc.named_scope("level_64"):
            tiles = emit_bias_gi4(0)
            for t in range(ARITY):
                ps_rz, ps_hn, ps_in = tiles

                def view3(pst):
                    return pst[:, :N3].rearrange("p (j n) -> p j n", j=3)

                for off, moff in ((0, 0), (N3, 3), (None, 6)):
                    pst, base = (ps_hn, 0) if off is None else (ps_rz, off)
                    for jo in range(J):
                        for ji in range(J):
                            nc.tensor.matmul(
                                pst[:, base + jo * P4 : base + (jo + 1) * P4],
                                whh_sb[:, ji, moff + jo, :],
                                h4[:, ji, :],
                                start=False,
                                stop=(jo == 2 and ji == 2 and moff != 0),
                            )
                if t + 1 < ARITY:
                    tiles = emit_bias_gi4(t + 1)

                rz_sb = gates.tile([P, 2, J, P4], bf16, name="rz4", tag="rz4")
                n_sb = gates.tile([P, J, P4], bf16, name="n4", tag="n4")
                rhn = gates.tile([P, J, P4], f32, name="rhn4", tag="rhn4")
                t1 = gates.tile([P, J, P4], bf16, name="t14", tag="t14")
                r_sb = rz_sb[:, 0]
                z_sb = rz_sb[:, 1]

                nc.scalar.activation(
                    rz_sb[:],
                    ps_rz[:, : 2 * N3].rearrange("p (r j n) -> p r j n", r=2, j=3),
                    Sig,
                )
                nc.vector.tensor_tensor(
                    out=rhn[:], in0=view3(ps_hn), in1=r_sb, op=Mult
                )
                nc.vector.tensor_tensor(
                    out=rhn[:], in0=rhn[:], in1=view3(ps_in), op=Add
                )
                nc.scalar.activation(n_sb[:], rhn[:], Tanh)
                nc.vector.tensor_tensor(out=t1[:], in0=h4[:], in1=n_sb[:], op=Sub)
                nc.vector.tensor_tensor(out=t1[:], in0=z_sb, in1=t1[:], op=Mult)
                nc.vector.tensor_tensor(out=h4[:], in0=n_sb[:], in1=t1[:], op=Add)
                # off the critical chain: gpsimd is idle during level 64
                nc.gpsimd.tensor_tensor(
                    out=hacc4[:], in0=hacc4[:], in1=h4[:], op=Add
                )

        # ---- outputs: raw h-sum (x3*8) and final hiddens of the 64 nodes ----
        nc.sync.dma_start(out_hacc[:], hacc4[:])
        nc.sync.dma_start(out_hf[:], h4[:])


def _build_program():
    if "prog" in _PROG_CACHE:
        return _PROG_CACHE["prog"]
    import concourse.bacc as bacc
    import concourse.mybir as mybir
    import concourse.tile as tile

    f32 = mybir.dt.float32
    bf16 = mybir.dt.bfloat16

    nc = bacc.Bacc(
        "TRN2",
        target_bir_lowering=False,
        debug=False,
        enable_asserts=False,
        num_devices=NCORES,
        num_swdge_queues=2,
    )
    tokens32 = nc.dram_tensor(
        "tokens32", [P, ARITY * GT], mybir.dt.int32, kind="ExternalInput"
    ).ap()
    embed = nc.dram_tensor("embed", [VOCAB, DIM], bf16, kind="ExternalInput").ap()
    wpack = nc.dram_tensor("wpack", [P, 3, J, 9, P], bf16, kind="ExternalInput").ap()
    biases = nc.dram_tensor("biases", [P, 12], f32, kind="ExternalInput").ap()
    bpack = nc.dram_tensor(
        "bpack", [3, 4 * P + 3 * 512], bf16, kind="ExternalInput"
    ).ap()
    bpack6 = nc.dram_tensor(
        "bpack6", [6, P + 6 * P4], bf16, kind="ExternalInput"
    ).ap()
    out_hacc = nc.dram_tensor("out_hacc", [P, J, P4], f32, kind="ExternalOutput").ap()
    out_hf = nc.dram_tensor("out_hf", [P, J, P4], bf16, kind="ExternalOutput").ap()

    with tile.TileContext(nc) as tc:
        _emit(tc, nc, (tokens32, embed, wpack, biases, bpack, bpack6, out_hacc, out_hf))
    nc.compile()
    _PROG_CACHE["prog"] = nc
    return nc


def _retile_weights(w):
    # w: [1152, 384] -> lhsT tiles [128(k_part), 3(k), 9(m), 128(m_col)] bf16
    wt = np.ascontiguousarray(w.T)  # [384, 1152]
    wt = wt.reshape(J, P, 9, P).transpose(1, 0, 2, 3)
    return np.ascontiguousarray(wt).astype(BF16)


def _prep_bias(b_ih, b_hh):
    biases = np.zeros((P, 12), np.float32)
    comb = (b_ih + b_hh).reshape(9, P)
    biases[:, 0:6] = comb[0:6].T
    biases[:, 6:9] = b_hh.reshape(9, P)[6:9].T
    biases[:, 9:12] = b_ih.reshape(9, P)[6:9].T
    return biases


def _prep_bias_mm(b_ih, b_hh):
    # lhsT[k, ro, q] = bias[q, 3*ro + k]: the K=3 bias matmul against the
    # one-hot rhs yields out[q, (j, n)] = bias[q, 3*ro + j].
    b = _prep_bias(b_ih, b_hh)  # [128, 12] cols: r0..2 z0..2 hn0..2 in0..2
    out = b.T.reshape(4, 3, P).transpose(1, 0, 2)
    return np.ascontiguousarray(out).astype(BF16)


def _prep_onehot3():
    out = np.zeros((3, 3, 512), np.float32)
    for k in range(3):
        out[k, k, :] = 1.0
    return out.astype(BF16)


def _prep_bpack6(b_ih, b_hh):
    # K=6 bias matmul for the merged r+z PSUM tile: lhsT rows are the six
    # combined bias vectors (r jo0..2, z jo0..2), rhs is a [6, 6, 64] one-hot.
    b = _prep_bias(b_ih, b_hh)  # [128, 12]
    lhs = b[:, 0:6].T.astype(np.float32)  # [6, 128]
    oh = np.zeros((6, 6, P4), np.float32)
    for k in range(6):
        oh[k, k, :] = 1.0
    out = np.concatenate([lhs, oh.reshape(6, 6 * P4)], axis=1)
    return np.ascontiguousarray(out).astype(BF16)


def _prep_tokens32(tokens_core):
    # int32 indirect-DMA tokens: col c*4+g, row p holds tokens[(g*128+p)*8+c]
    tok = tokens_core.reshape(P5, ARITY).T  # [8 child, 512 parent]
    sel = tok.reshape(ARITY, GT, P).transpose(2, 0, 1).reshape(P, ARITY * GT)
    return np.ascontiguousarray(sel)


def _gru_level(x_children, h0, w_ih, w_hh, b_ih, b_hh):
    # x_children: [A, N, D] in original child order; consumed reversed.
    h = h0
    acc = np.zeros_like(h)
    for t in range(ARITY):
        x_t = x_children[ARITY - 1 - t]
        gi = x_t @ w_ih.T + b_ih
        gh = h @ w_hh.T + b_hh
        i_r, i_z, i_n = np.split(gi, 3, axis=-1)
        h_r, h_z, h_n = np.split(gh, 3, axis=-1)
        r = 1.0 / (1.0 + np.exp(-(i_r + h_r)))
        z = 1.0 / (1.0 + np.exp(-(i_z + h_z)))
        n = np.tanh(i_n + r * h_n)
        h = (1.0 - z) * n + z * h
        acc += h
    return acc / ARITY, h


def kernel(leaf_tokens, embed_table, w_ih, w_hh, b_ih, b_hh):
    from concourse.bass_utils import run_bass_kernel_spmd

    leaf_tokens = np.asarray(leaf_tokens, np.int32)
    embed_table = np.asarray(embed_table, np.float32)
    w_ih = np.asarray(w_ih, np.float32)
    w_hh = np.asarray(w_hh, np.float32)
    b_ih = np.asarray(b_ih, np.float32)
    b_hh = np.asarray(b_hh, np.float32)

    nc = _build_program()

    embed_bf = embed_table.astype(BF16)
    wpack = np.ascontiguousarray(
        np.stack(
            [
                _retile_weights(w_ih),
                _retile_weights(w_hh),
                _retile_weights(w_ih / ARITY),
            ],
            axis=1,
        )
    )
    biases = _prep_bias(b_ih, b_hh)
    bpack = np.ascontiguousarray(
        np.concatenate(
            [
                _prep_bias_mm(b_ih, b_hh).reshape(3, 4 * P),
                _prep_onehot3().reshape(3, 3 * 512),
            ],
            axis=1,
        )
    )
    in_maps = []
    for core in range(NCORES):
        in_maps.append(
            {
                "tokens32": _prep_tokens32(
                    leaf_tokens[core * LEAVES_CORE : (core + 1) * LEAVES_CORE]
                ),
                "embed": embed_bf,
                "wpack": wpack,
                "biases": biases,
                "bpack": bpack,
                "bpack6": _prep_bpack6(b_ih, b_hh),
            }
        )
    res = run_bass_kernel_spmd(nc, in_maps, core_ids=list(range(NCORES)))

    # device tensors -> [core, 64 nodes, 384] with f = j*128 + p
    x3 = np.zeros((NCORES, P4, DIM), np.float64)
    h3 = np.zeros((NCORES, P4, DIM), np.float64)
    for core in range(NCORES):
        hacc = np.asarray(res.results[core]["out_hacc"], np.float64)  # [128,3,64]
        hf = np.asarray(res.results[core]["out_hf"], np.float64)
        x3[core] = (hacc / ARITY).transpose(1, 0, 2).reshape(DIM, P4).T
        h3[core] = hf.transpose(1, 0, 2).reshape(DIM, P4).T

    w_ih64 = w_ih.astype(np.float64)
    w_hh64 = w_hh.astype(np.float64)
    b_ih64 = b_ih.astype(np.float64)
    b_hh64 = b_hh.astype(np.float64)

    # level 3: per core, 8 parents x 8 children (batch over cores*parents)
    xc = x3.reshape(NCORES * ARITY, ARITY, DIM).transpose(1, 0, 2)  # [A, 64, D]
    h0 = h3.reshape(NCORES * ARITY, ARITY, DIM).mean(axis=1)
    x2, h2 = _gru_level(xc, h0, w_ih64, w_hh64, b_ih64, b_hh64)

    # level 2: per core, 1 parent x 8 children
    xc = x2.reshape(NCORES, ARITY, DIM).transpose(1, 0, 2)  # [A, 8, D]
    h0 = h2.reshape(NCORES, ARITY, DIM).mean(axis=1)
    x1, h1 = _gru_level(xc, h0, w_ih64, w_hh64, b_ih64, b_hh64)

    # root: 8 cores' outputs
    xc = x1.reshape(1, ARITY, DIM).transpose(1, 0, 2)  # [A, 1, D]
    h0 = h1.reshape(1, ARITY, DIM).mean(axis=1)
    out, _ = _gru_level(xc, h0, w_ih64, w_hh64, b_ih64, b_hh64)

    return out.astype(np.float32).reshape(1, 1, DIM)


# revision 62
# speedup vs baseline: 1.2462x; 1.0178x over previous
"""Tree-GRU (arity-8, depth-5) over embedded leaves on 8 TRN2 NeuronCores.

Sharding: data-parallel over subtrees. Each core takes 4096 contiguous leaves
and runs levels 5 and 4 of the tree locally (512 -> 64 parents). The last two
per-core levels (64 -> 8 -> 1) and the root are small latency-bound GRU
cascades (free dim <= 8) done on host in fp64 after gathering the per-core
level-4 outputs, extending the baseline's host-side root reduction.

Device layout is feature-transposed: tensors live as [128 part, 3 ktile, ...]
with feature f = 128*k + p, so the GRU matmuls contract the partition dim.

Embeddings arrive per GRU step: tokens are host-permuted child-major, each
child's 512 rows fetched by 4 indirect DMAs (leaf-major) and flipped
feature-major by 4 xbar transpose-DMAs on the HWDGE rings — no tensor-engine
transposes, no PSUM, and the first GRU matmul can start after ~2 gathers.

Level 512 keeps one PSUM bank per (role, jo) output tile at N=512; each step
emits gi matmuls of units j0/j1 ahead of any hh matmul so the tensor engine
holds ~3.8us of h-independent work to hide the previous step's gate chain.
Unit j1 owns 4 banks (double-buffered step to step); j0 and j2 share the
other 4, with j2's allocation waiting on j0's progressively-freed banks
behind hh j1. Biases ride the scalar-activation bias port. The per-step
output accumulator is kept child-major (vector engine) so the final step
writes level 4's input directly as a fused raw-sum add; the 1/8 output-mean
scale is folded into a pre-scaled copy of W_ih used by level 4. Level 64
injects biases into PSUM via a K=3 one-hot matmul (the only start=True
write), collapsing the gate chain to jo-spanning instructions.
"""

import numpy as np
import ml_dtypes

ARITY = 8
DIM = 384
VOCAB = 32000
NCORES = 8
P = 128
J = 3  # DIM // 128 feature tiles
N_LEAVES = 32768
LEAVES_CORE = N_LEAVES // NCORES  # 4096
P5 = LEAVES_CORE // ARITY  # 512 level-5 parents per core
P4 = P5 // ARITY  # 64 level-4 parents per core
GT = P5 // P  # 4 gather tiles per child

BF16 = ml_dtypes.bfloat16

_PROG_CACHE = {}


def _emit(tc, nc, aps):
    import concourse.mybir as mybir
    import concourse.bass as bass
    from concourse.masks import make_identity

    f32 = mybir.dt.float32
    bf16 = mybir.dt.bfloat16
    Sig = mybir.ActivationFunctionType.Sigmoid
    Tanh = mybir.ActivationFunctionType.Tanh
    Add = mybir.AluOpType.add
    Sub = mybir.AluOpType.subtract
    Mult = mybir.AluOpType.mult

    tokens32, embed, wih_t, biases, biases_mm, bpack6, out_hacc, out_hf = aps

    from contextlib import ExitStack

    with ExitStack() as ctx:
        const = ctx.enter_context(tc.tile_pool(name="const", bufs=1))
        xpool = ctx.enter_context(tc.tile_pool(name="xpool", bufs=1))
        gpool = ctx.enter_context(tc.tile_pool(name="gpool", bufs=3))
        state = ctx.enter_context(tc.tile_pool(name="state", bufs=1))
        gates = ctx.enter_context(tc.tile_pool(name="gates", bufs=4))
        pspool = ctx.enter_context(tc.tile_pool(name="pspool", bufs=4, space="PSUM"))
        pspool2 = ctx.enter_context(tc.tile_pool(name="pspool2", bufs=4, space="PSUM"))

        # ---- token tile first, then per-child embedding gathers ----
        # all children are fetched leaf-major by native indirect DMAs (no
        # GpSimd library, starts right after the token DMA) and flipped
        # feature-major by tensor-engine transposes two steps ahead of use.
        tok32_sb = const.tile([P, ARITY * GT], mybir.dt.int32)
        nc.sync.dma_start(tok32_sb[:], tokens32[:])

        wpack_sb = const.tile([P, 3, J, 9, P], bf16)
        wih_sb = wpack_sb[:, 0]
        whh_sb = wpack_sb[:, 1]
        wih_s_sb = wpack_sb[:, 2]
        bias_sb = const.tile([P, 12], f32)
        bpack_sb = const.tile([3, 4 * P + 3 * 512], bf16)
        bias3_sb = bpack_sb[:, : 4 * P].rearrange("k (r p) -> k r p", r=4)
        onehot3_sb = bpack_sb[:, 4 * P :].rearrange("k (j n) -> k j n", j=3)
        bpack6_sb = const.tile([6, P + 6 * P4], bf16)
        bias6_sb = bpack6_sb[:, :P]
        onehot6_sb = bpack6_sb[:, P:].rearrange("k (g n) -> k g n", g=6)
        nc.sync.dma_start(wpack_sb[:], wih_t[:])
        nc.sync.dma_start(bias_sb[:], biases[:])
        nc.sync.dma_start(bpack_sb[:], biases_mm[:])
        nc.sync.dma_start(bpack6_sb[:], bpack6[:])

        # x5[p, child, j, q]
        x5 = xpool.tile([P, ARITY, J, P5], bf16, name="x5", tag="x5")
        ident = const.tile([P, P], bf16)
        make_identity(nc, ident[:])

        xgs = {}
        for t in range(ARITY):
            c = ARITY - 1 - t  # children consumed in reverse: child 7 first
            xg = gpool.tile([P, GT, DIM], bf16, name="xg", tag="xg")
            xgs[c] = xg
            for g in range(GT):
                gi_inst = nc.gpsimd.indirect_dma_start(
                    out=xg[:, g, :],
                    out_offset=None,
                    in_=embed[:],
                    in_offset=bass.IndirectOffsetOnAxis(
                        ap=tok32_sb[:, c * GT + g : c * GT + g + 1], axis=0
                    ),
                )
                if (t * GT + g) % 2 == 1:
                    gi_inst.ins.queue = "qPoolDynamic1"

        def emit_transposes(c):
            # 4 gather tiles of one feature third -> one PSUM bank, one copy
            xg = xgs[c]
            for j in range(J):
                tp = pspool2.tile([P, 512], bf16, name="tp", tag="ps2")
                for g in range(GT):
                    nc.tensor.transpose(
                        tp[:, g * P : (g + 1) * P],
                        xg[:, g, j * P : (j + 1) * P],
                        ident[:],
                    )
                nc.vector.tensor_copy(out=x5[:, c, j, :], in_=tp[:])

        emit_transposes(7)
        emit_transposes(6)

        x4 = xpool.tile([P, ARITY, J, P4], bf16, name="x4", tag="x4")

        def psum_tile(jo):
            # 8 banks for 12 role-tiles per step: unit j1 owns pspool (reuse
            # waits on the previous step's j1 gates); j0/j2 share pspool2 —
            # j2 waits on same-step j0 gates (freed progressively under hh
            # j1), j0 on the previous step's j2 gates. All waits point at
            # strictly earlier FIFO positions: no deadlock.
            if jo == 1:
                return pspool.tile([P, 512], f32, name="ps", tag="ps")
            return pspool2.tile([P, 512], f32, name="ps2", tag="ps2")

        # =================== level 5: 512 parents, leaf children ===================
        h5 = state.tile([P, J, P5], bf16, name="h5", tag="h5")
        hacc5 = state.tile([P, J, P5], f32, name="hacc5", tag="hacc5")
        nc.gpsimd.memset(hacc5[:], 0.0)
        csum5 = state.tile([P, J, P4], f32, name="csum5", tag="csum5")

        with nc.named_scope("level_512"):
            for t in range(ARITY):
                c = ARITY - 1 - t
                leaf0 = t == 0

                ps_r = [None] * J
                ps_z = [None] * J
                ps_in = [None] * J
                ps_hn = [None] * J

                def emit_gi(jo):
                    ps_r[jo] = psum_tile(jo)
                    ps_z[jo] = psum_tile(jo)
                    ps_in[jo] = psum_tile(jo)
                    if not leaf0:
                        ps_hn[jo] = psum_tile(jo)
                    for ps, moff in ((ps_r[jo], 0), (ps_z[jo], 3), (ps_in[jo], 6)):
                        for ji in range(J):
                            nc.tensor.matmul(
                                ps[:, :P5],
                                wih_sb[:, ji, moff + jo, :],
                                x5[:, c, ji, :],
                                start=(ji == 0),
                                stop=(ji == 2 and (moff == 6 or leaf0)),
                            )

                def emit_hh(jo):
                    if leaf0:
                        return
                    for ps, moff in ((ps_r[jo], 0), (ps_z[jo], 3), (ps_hn[jo], 6)):
                        for ji in range(J):
                            nc.tensor.matmul(
                                ps[:, :P5],
                                whh_sb[:, ji, moff + jo, :],
                                h5[:, ji, :],
                                start=(ji == 0 and moff == 6),
                                stop=(ji == 2),
                            )

                emit_gi(0)
                emit_gi(1)
                if t <= 5:
                    # next-next child's feature flip rides behind ~3.8us of
                    # gi matmuls so its PSUM-bank wait never stalls the PE
                    emit_transposes(5 - t)
                emit_hh(0)
                emit_hh(1)
                emit_gi(2)
                emit_hh(2)

                for jo in range(J):
                    r_sb = gates.tile([P, P5], bf16, name="r_sb", tag="r_sb")
                    z_sb = gates.tile([P, P5], bf16, name="z_sb", tag="z_sb")
                    n_sb = gates.tile([P, P5], bf16, name="n_sb", tag="n_sb")
                    rhn = gates.tile([P, P5], f32, name="rhn", tag="rhn")
                    t1 = gates.tile([P, P5], bf16, name="t1", tag="t1")

                    nc.scalar.activation(
                        r_sb[:], ps_r[jo][:, :P5], Sig, bias=bias_sb[:, jo : jo + 1]
                    )
                    nc.scalar.activation(
                        z_sb[:], ps_z[jo][:, :P5], Sig, bias=bias_sb[:, 3 + jo : 4 + jo]
                    )
                    if leaf0:
                        nc.vector.tensor_scalar_mul(
                            rhn[:], r_sb[:], bias_sb[:, 6 + jo : 7 + jo]
                        )
                    else:
                        nc.vector.scalar_tensor_tensor(
                            out=rhn[:],
                            in0=ps_hn[jo][:, :P5],
                            scalar=bias_sb[:, 6 + jo : 7 + jo],
                            in1=r_sb[:],
                            op0=Add,
                            op1=Mult,
                        )
                    nc.vector.tensor_tensor(
                        out=rhn[:], in0=rhn[:], in1=ps_in[jo][:, :P5], op=Add
                    )
                    nc.scalar.activation(
                        n_sb[:], rhn[:], Tanh, bias=bias_sb[:, 9 + jo : 10 + jo]
                    )
                    hsl = h5[:, jo, :]
                    if leaf0:
                        nc.vector.tensor_tensor(out=t1[:], in0=z_sb[:], in1=n_sb[:], op=Mult)
                        nc.vector.tensor_tensor(out=hsl, in0=n_sb[:], in1=t1[:], op=Sub)
                    else:
                        nc.vector.tensor_tensor(out=t1[:], in0=hsl, in1=n_sb[:], op=Sub)
                        nc.vector.tensor_tensor(out=t1[:], in0=z_sb[:], in1=t1[:], op=Mult)
                        nc.vector.tensor_tensor(out=hsl, in0=n_sb[:], in1=t1[:], op=Add)

                # output accumulation after the chain ops so it never delays
                # the next step's recurrent matmuls
                if t < ARITY - 1:
                    for jo in range(J):
                        # contiguous accumulate: strided reads here would
                        # triple the op cost and starve the gate chain
                        nc.vector.tensor_tensor(
                            out=hacc5[:, jo], in0=hacc5[:, jo], in1=h5[:, jo, :], op=Add
                        )
                else:
                    # all child-mean reduces first: they gate h4 and with it
                    # every hh matmul of level 64
                    for jo in range(J):
                        nc.vector.tensor_reduce(
                            out=csum5[:, jo, :],
                            in_=h5[:, jo, :].rearrange("p (q c) -> p q c", c=ARITY),
                            axis=mybir.AxisListType.X,
                            op=Add,
                        )
                    # the only permuted pass: child-major writes, child 7
                    # complete after three ops so level 64's first gi matmuls
                    # unblock early
                    for cc in range(ARITY - 1, -1, -1):
                        for jo in range(J):
                            nc.vector.tensor_tensor(
                                out=x4[:, cc, jo, :],
                                in0=hacc5[:, jo].rearrange(
                                    "p (q c) -> p c q", c=ARITY
                                )[:, cc],
                                in1=h5[:, jo, :].rearrange(
                                    "p (q c) -> p c q", c=ARITY
                                )[:, cc],
                                op=Add,
                            )

        # =================== level 4: 64 parents ===================
        h4 = state.tile([P, J, P4], bf16, name="h4", tag="h4")
        nc.scalar.mul(h4[:], csum5[:], 1.0 / ARITY)
        hacc4 = state.tile([P, J, P4], f32, name="hacc4", tag="hacc4")
        nc.gpsimd.memset(hacc4[:], 0.0)
        N3 = J * P4  # 192

        def emit_bias_gi4(t):
            # bias matmuls are the only start=True writes; r and z share one
            # tile via a K=6 one-hot so a single sigmoid covers both. Emitted
            # one step ahead (x4 and the previous banks are ready) so these
            # ~30 matmuls keep the tensor engine busy under the gate chain.
            c = ARITY - 1 - t
            pool = pspool if t % 2 == 0 else pspool2
            tag = "ps" if t % 2 == 0 else "ps2"
            ps_rz, ps_hn, ps_in = (
                pool.tile([P, 512], f32, name=tag, tag=tag) for _ in range(3)
            )
            nc.tensor.matmul(
                ps_rz[:, : 2 * N3],
                bias6_sb[:],
                onehot6_sb[:],
                start=True,
                stop=False,
            )
            for pst, ro in ((ps_hn, 2), (ps_in, 3)):
                nc.tensor.matmul(
                    pst[:, :N3],
                    bias3_sb[:, ro, :],
                    onehot3_sb[:, :, :P4],
                    start=True,
                    stop=False,
                )
            for off, moff in ((0, 0), (N3, 3), (None, 6)):
                pst, base = (ps_in, 0) if off is None else (ps_rz, off)
                for jo in range(J):
                    for ji in range(J):
                        nc.tensor.matmul(
                            pst[:, base + jo * P4 : base + (jo + 1) * P4],
                            wih_s_sb[:, ji, moff + jo, :],
                            x4[:, c, ji, :],
                            start=False,
                            stop=(moff == 6 and jo == 2 and ji == 2),
                        )
            return ps_rz, ps_hn, ps_in

        with nc.named_scope("level_64"):
            tiles = emit_bias_gi4(0)
            for t in range(ARITY):
                ps_rz, ps_hn, ps_in = tiles

                def view3(pst):
                    return pst[:, :N3].rearrange("p (j n) -> p j n", j=3)

                for off, moff in ((0, 0), (N3, 3), (None, 6)):
                    pst, base = (ps_hn, 0) if off is None else (ps_rz, off)
                    for jo in range(J):
                        for ji in range(J):
                            nc.tensor.matmul(
                                pst[:, base + jo * P4 : base + (jo + 1) * P4],
                                whh_sb[:, ji, moff + jo, :],
                                h4[:, ji, :],
                                start=False,
                                stop=(jo == 2 and ji == 2 and moff != 0),
                            )
                if t + 1 < ARITY:
                    tiles = emit_bias_gi4(t + 1)

                rz_sb = gates.tile([P, 2, J, P4], bf16, name="rz4", tag="rz4")
                n_sb = gates.tile([P, J, P4], bf16, name="n4", tag="n4")
                rhn = gates.tile([P, J, P4], f32, name="rhn4", tag="rhn4")
                t1 = gates.tile([P, J, P4], bf16, name="t14", tag="t14")
                r_sb = rz_sb[:, 0]
                z_sb = rz_sb[:, 1]

                nc.scalar.activation(
                    rz_sb[:],
                    ps_rz[:, : 2 * N3].rearrange("p (r j n) -> p r j n", r=2, j=3),
                    Sig,
                )
                nc.vector.tensor_tensor(
                    out=rhn[:], in0=view3(ps_hn), in1=r_sb, op=Mult
                )
                nc.vector.tensor_tensor(
                    out=rhn[:], in0=rhn[:], in1=view3(ps_in), op=Add
                )
                nc.scalar.activation(n_sb[:], rhn[:], Tanh)
                nc.vector.tensor_tensor(out=t1[:], in0=h4[:], in1=n_sb[:], op=Sub)
                nc.vector.tensor_tensor(out=t1[:], in0=z_sb, in1=t1[:], op=Mult)
                nc.vector.tensor_tensor(out=h4[:], in0=n_sb[:], in1=t1[:], op=Add)
                # off the critical chain: gpsimd is idle during level 64
                nc.gpsimd.tensor_tensor(
                    out=hacc4[:], in0=hacc4[:], in1=h4[:], op=Add
                )

        # ---- outputs: raw h-sum (x3*8) and final hiddens of the 64 nodes ----
        nc.sync.dma_start(out_hacc[:], hacc4[:])
        nc.sync.dma_start(out_hf[:], h4[:])


def _build_program():
    if "prog" in _PROG_CACHE:
        return _PROG_CACHE["prog"]
    import concourse.bacc as bacc
    import concourse.mybir as mybir
    import concourse.tile as tile

    f32 = mybir.dt.float32
    bf16 = mybir.dt.bfloat16

    nc = bacc.Bacc(
        "TRN2",
        target_bir_lowering=False,
        debug=False,
        enable_asserts=False,
        num_devices=NCORES,
        num_swdge_queues=2,
    )
    tokens32 = nc.dram_tensor(
        "tokens32", [P, ARITY * GT], mybir.dt.int32, kind="ExternalInput"
    ).ap()
    embed = nc.dram_tensor("embed", [VOCAB, DIM], bf16, kind="ExternalInput").ap()
    wpack = nc.dram_tensor("wpack", [P, 3, J, 9, P], bf16, kind="ExternalInput").ap()
    biases = nc.dram_tensor("biases", [P, 12], f32, kind="ExternalInput").ap()
    bpack = nc.dram_tensor(
        "bpack", [3, 4 * P + 3 * 512], bf16, kind="ExternalInput"
    ).ap()
    bpack6 = nc.dram_tensor(
        "bpack6", [6, P + 6 * P4], bf16, kind="ExternalInput"
    ).ap()
    out_hacc = nc.dram_tensor("out_hacc", [P, J, P4], f32, kind="ExternalOutput").ap()
    out_hf = nc.dram_tensor("out_hf", [P, J, P4], bf16, kind="ExternalOutput").ap()

    with tile.TileContext(nc) as tc:
        _emit(tc, nc, (tokens32, embed, wpack, biases, bpack, bpack6, out_hacc, out_hf))
    nc.compile()
    _PROG_CACHE["prog"] = nc
    return nc


def _retile_weights(w):
    # w: [1152, 384] -> lhsT tiles [128(k_part), 3(k), 9(m), 128(m_col)] bf16
    wt = np.ascontiguousarray(w.T)  # [384, 1152]
    wt = wt.reshape(J, P, 9, P).transpose(1, 0, 2, 3)
    return np.ascontiguousarray(wt).astype(BF16)


def _prep_bias(b_ih, b_hh):
    biases = np.zeros((P, 12), np.float32)
    comb = (b_ih + b_hh).reshape(9, P)
    biases[:, 0:6] = comb[0:6].T
    biases[:, 6:9] = b_hh.reshape(9, P)[6:9].T
    biases[:, 9:12] = b_ih.reshape(9, P)[6:9].T
    return biases


def _prep_bias_mm(b_ih, b_hh):
    # lhsT[k, ro, q] = bias[q, 3*ro + k]: the K=3 bias matmul against the
    # one-hot rhs yields out[q, (j, n)] = bias[q, 3*ro + j].
    b = _prep_bias(b_ih, b_hh)  # [128, 12] cols: r0..2 z0..2 hn0..2 in0..2
    out = b.T.reshape(4, 3, P).transpose(1, 0, 2)
    return np.ascontiguousarray(out).astype(BF16)


def _prep_onehot3():
    out = np.zeros((3, 3, 512), np.float32)
    for k in range(3):
        out[k, k, :] = 1.0
    return out.astype(BF16)


def _prep_bpack6(b_ih, b_hh):
    # K=6 bias matmul for the merged r+z PSUM tile: lhsT rows are the six
    # combined bias vectors (r jo0..2, z jo0..2), rhs is a [6, 6, 64] one-hot.
    b = _prep_bias(b_ih, b_hh)  # [128, 12]
    lhs = b[:, 0:6].T.astype(np.float32)  # [6, 128]
    oh = np.zeros((6, 6, P4), np.float32)
    for k in range(6):
        oh[k, k, :] = 1.0
    out = np.concatenate([lhs, oh.reshape(6, 6 * P4)], axis=1)
    return np.ascontiguousarray(out).astype(BF16)


def _prep_tokens32(tokens_core):
    # int32 indirect-DMA tokens: col c*4+g, row p holds tokens[(g*128+p)*8+c]
    tok = tokens_core.reshape(P5, ARITY).T  # [8 child, 512 parent]
    sel = tok.reshape(ARITY, GT, P).transpose(2, 0, 1).reshape(P, ARITY * GT)
    return np.ascontiguousarray(sel)


def _gru_level(x_children, h0, w_ih, w_hh, b_ih, b_hh):
    # x_children: [A, N, D] in original child order; consumed reversed.
    h = h0
    acc = np.zeros_like(h)
    for t in range(ARITY):
        x_t = x_children[ARITY - 1 - t]
        gi = x_t @ w_ih.T + b_ih
        gh = h @ w_hh.T + b_hh
        i_r, i_z, i_n = np.split(gi, 3, axis=-1)
        h_r, h_z, h_n = np.split(gh, 3, axis=-1)
        r = 1.0 / (1.0 + np.exp(-(i_r + h_r)))
        z = 1.0 / (1.0 + np.exp(-(i_z + h_z)))
        n = np.tanh(i_n + r * h_n)
        h = (1.0 - z) * n + z * h
        acc += h
    return acc / ARITY, h


def kernel(leaf_tokens, embed_table, w_ih, w_hh, b_ih, b_hh):
    from concourse.bass_utils import run_bass_kernel_spmd

    leaf_tokens = np.asarray(leaf_tokens, np.int32)
    embed_table = np.asarray(embed_table, np.float32)
    w_ih = np.asarray(w_ih, np.float32)
    w_hh = np.asarray(w_hh, np.float32)
    b_ih = np.asarray(b_ih, np.float32)
    b_hh = np.asarray(b_hh, np.float32)

    nc = _build_program()

    embed_bf = embed_table.astype(BF16)
    wpack = np.ascontiguousarray(
        np.stack(
            [
                _retile_weights(w_ih),
                _retile_weights(w_hh),
                _retile_weights(w_ih / ARITY),
            ],
            axis=1,
        )
    )
    biases = _prep_bias(b_ih, b_hh)
    bpack = np.ascontiguousarray(
        np.concatenate(
            [
                _prep_bias_mm(b_ih, b_hh).reshape(3, 4 * P),
                _prep_onehot3().reshape(3, 3 * 512),
            ],
            axis=1,
        )
    )
    in_maps = []
    for core in range(NCORES):
        in_maps.append(
            {
                "tokens32": _prep_tokens32(
                    leaf_tokens[core * LEAVES_CORE : (core + 1) * LEAVES_CORE]
                ),
                "embed": embed_bf,
                "wpack": wpack,
                "biases": biases,
                "bpack": bpack,
                "bpack6": _prep_bpack6(b_ih, b_hh),
            }
        )
    res = run_bass_kernel_spmd(nc, in_maps, core_ids=list(range(NCORES)))

    # device tensors -> [core, 64 nodes, 384] with f = j*128 + p
    x3 = np.zeros((NCORES, P4, DIM), np.float64)
    h3 = np.zeros((NCORES, P4, DIM), np.float64)
    for core in range(NCORES):
        hacc = np.asarray(res.results[core]["out_hacc"], np.float64)  # [128,3,64]
        hf = np.asarray(res.results[core]["out_hf"], np.float64)
        x3[core] = (hacc / ARITY).transpose(1, 0, 2).reshape(DIM, P4).T
        h3[core] = hf.transpose(1, 0, 2).reshape(DIM, P4).T

    w_ih64 = w_ih.astype(np.float64)
    w_hh64 = w_hh.astype(np.float64)
    b_ih64 = b_ih.astype(np.float64)
    b_hh64 = b_hh.astype(np.float64)

    # level 3: per core, 8 parents x 8 children (batch over cores*parents)
    xc = x3.reshape(NCORES * ARITY, ARITY, DIM).transpose(1, 0, 2)  # [A, 64, D]
    h0 = h3.reshape(NCORES * ARITY, ARITY, DIM).mean(axis=1)
    x2, h2 = _gru_level(xc, h0, w_ih64, w_hh64, b_ih64, b_hh64)

    # level 2: per core, 1 parent x 8 children
    xc = x2.reshape(NCORES, ARITY, DIM).transpose(1, 0, 2)  # [A, 8, D]
    h0 = h2.reshape(NCORES, ARITY, DIM).mean(axis=1)
    x1, h1 = _gru_level(xc, h0, w_ih64, w_hh64, b_ih64, b_hh64)

    # root: 8 cores' outputs
    xc = x1.reshape(1, ARITY, DIM).transpose(1, 0, 2)  # [A, 1, D]
    h0 = h1.reshape(1, ARITY, DIM).mean(axis=1)
    out, _ = _gru_level(xc, h0, w_ih64, w_hh64, b_ih64, b_hh64)

    return out.astype(np.float32).reshape(1, 1, DIM)
